# revision 1
# baseline (speedup 1.0000x reference)
"""GQA attention with ALiBi (non-causal) on 8 TRN2 NeuronCores.

Sharding: 8 cores = 4 batches x 2 query-halves. Each core computes all 16
heads for its 1024 queries. Key insight: without a causal mask the ALiBi bias
slope_h*(j-i) is, inside the softmax over j, equivalent to a per-column bias
slope_h*j (the -slope_h*i term is constant per row and cancels). Hence
attention weights concentrate on the last keys and each head only needs the
trailing window of keys where exp(slope_h*(j - (S-1))) is non-negligible.

Device dataflow (transpose-free):
  q^T [heads*hd, q]   = Wq^T @ x^T          (lhsT=Wq, rhs=x^T)
  k^T [kv*hd, keys]   = Wk^T @ x^T          (windowed keys)
  v   [keys, kv*hd]   = x @ Wv              (lhsT=x^T chunk, rhs=Wv)
  S^T [keys, q]       = k^T.T-chunk @ q^T   (2 heads packed via PE row tiling)
  P^T = exp(S^T + lnc[key])                 (ALiBi factor as per-partition ACT bias)
  out^T [hd+1, q]    += vext^T-chunk @ P^T  (vext = [v | 1]; row hd = softmax denom)
  y^T [D, q]          = Wo^T @ (out^T/den)
Host returns y = y^T.T per core, concatenated.
"""
import math
import os
from contextlib import ExitStack

import numpy as np

B, S, D = 4, 2048, 1024
H, KV, HD = 16, 4, 64
GROUPS = H // KV
N_CORES = 8
QH = S // 2          # queries per core
CH = 128             # key chunk (PE contraction tile)
NCH = S // CH        # 16 chunks
MARGIN = float(os.environ.get("KERNEL_MARGIN", "14.0"))

LAST_RESULT = None   # BassKernelResults of the most recent run (for profiling)


def _slopes():
    start = 2.0 ** (-(2.0 ** -(math.log2(H) - 3)))
    return np.array([start * start**i for i in range(H)], dtype=np.float64)


SLOPES = _slopes()
# chunks of trailing keys needed per head / kv-group
CHUNKS_H = [min(NCH, max(1, int(math.ceil(MARGIN / s / CH)))) for s in SLOPES]
CHUNKS_G = [CHUNKS_H[4 * g + 3] for g in range(KV)]

# lnc table: one column per (head, chunk) = slope_h * (j - (S-1))
_ENTRIES = {}
for _h in range(H):
    for _c in range(NCH - CHUNKS_H[_h], NCH):
        _ENTRIES[(_h, _c)] = len(_ENTRIES)
N_ENT = len(_ENTRIES)


def _lnc_table():
    t = np.zeros((CH, N_ENT), dtype=np.float32)
    for (h, c), e in _ENTRIES.items():
        j = c * CH + np.arange(CH, dtype=np.float64)
        t[:, e] = (SLOPES[h] * (j - (S - 1))).astype(np.float32)
    return t


_NC_CACHE = None


def _build():
    import concourse.bass as bass
    import concourse.tile as tile
    from concourse import bacc, mybir
    from concourse.bass_interp import get_hw_module

    f32 = mybir.dt.float32
    f32r = mybir.dt.float32r
    Exp = mybir.ActivationFunctionType.Exp

    nc = bacc.Bacc("TRN2", target_bir_lowering=False, debug=False,
                   num_devices=N_CORES)
    xt_d = nc.dram_tensor("xt", [D, S], f32r, kind="ExternalInput").ap()
    xq_d = nc.dram_tensor("xq", [D, QH], f32r, kind="ExternalInput").ap()
    wq_d = nc.dram_tensor("wq", [D, D], f32r, kind="ExternalInput").ap()
    wk_d = nc.dram_tensor("wk", [D, KV * HD], f32r, kind="ExternalInput").ap()
    wv_d = nc.dram_tensor("wv", [D, KV * HD], f32r, kind="ExternalInput").ap()
    wo_d = nc.dram_tensor("wo", [D, D], f32r, kind="ExternalInput").ap()
    lnc_d = nc.dram_tensor("lnc", [CH, N_ENT], f32, kind="ExternalInput").ap()
    ones_d = nc.dram_tensor("ones", [CH, NCH], f32r, kind="ExternalInput").ap()
    yt_d = nc.dram_tensor("yt", [D, QH], f32, kind="ExternalOutput").ap()

    with tile.TileContext(nc) as tc, ExitStack() as ctx:
        persist = ctx.enter_context(tc.tile_pool(name="persist", bufs=1))
        lnc_sb = persist.tile([CH, N_ENT], f32)
        nc.sync.dma_start(out=lnc_sb[:], in_=lnc_d[:])
        qt = [persist.tile([128, QH], f32r, tag=f"qt{p}", name=f"qt{p}") for p in range(8)]
        kdup = [persist.tile([128, CHUNKS_G[g] * CH], f32r, tag=f"kd{g}", name=f"kd{g}")
                for g in range(KV)]
        vext = [persist.tile([128, CHUNKS_G[g], HD + 1], f32r, tag=f"ve{g}", name=f"ve{g}")
                for g in range(KV)]
        outst = [persist.tile([128, QH], f32r, tag=f"os{p}", name=f"os{p}") for p in range(8)]

        # ---------------- phase A: projections ----------------
        with ExitStack() as pctx:
            xw = pctx.enter_context(tc.tile_pool(name="xw", bufs=1))
            xq_sb = xw.tile([128, 8, QH], f32r)
            nc.sync.dma_start(out=xq_sb[:],
                              in_=xq_d.rearrange("(k p) s -> p k s", p=128))
            wkv_sb = xw.tile([128, 8, 2 * KV * HD], f32r)
            nc.sync.dma_start(out=wkv_sb[:, :, 0:KV * HD],
                              in_=wk_d.rearrange("(k p) c -> p k c", p=128))
            nc.sync.dma_start(out=wkv_sb[:, :, KV * HD:],
                              in_=wv_d.rearrange("(k p) c -> p k c", p=128))
            wqs = pctx.enter_context(tc.tile_pool(name="wqs", bufs=2))
            xts = pctx.enter_context(tc.tile_pool(name="xts", bufs=2))
            wq_r = wq_d.rearrange("(k p) c -> p k c", p=128)
            xt_r = xt_d.rearrange("(k p) s -> p k s", p=128)

            qp = pctx.enter_context(tc.tile_pool(name="qp", bufs=2, space="PSUM"))
            kp = pctx.enter_context(tc.tile_pool(name="kp", bufs=2, space="PSUM"))

            # q^T: per pair-of-heads m-tile (wq streamed per m-tile)
            for mt in range(8):
                wq_t = wqs.tile([128, 8, 128], f32r, tag="wq")
                nc.sync.dma_start(out=wq_t[:],
                                  in_=wq_r[:, :, mt * 128:(mt + 1) * 128])
                ps = qp.tile([128, QH], f32, tag="qps")
                for k in range(8):
                    for qc in range(2):
                        nc.tensor.matmul(
                            ps[:, qc * 512:(qc + 1) * 512],
                            (wq_t[:, k, :]),
                            (xq_sb[:, k, qc * 512:(qc + 1) * 512]),
                            start=(k == 0), stop=(k == 7))
                nc.vector.tensor_copy(qt[mt][:], ps[:])

            # k^T and v, streaming x^T per 512-key block
            for i5 in (3, 2, 1, 0):
                key0 = i5 * 512
                xt_t = xts.tile([128, 8, 512], f32r, tag="xt")
                nc.sync.dma_start(out=xt_t[:], in_=xt_r[:, :, key0:key0 + 512])
                # k^T m-tiles whose window intersects this block
                for mt in range(2):
                    w0 = S - CHUNKS_G[2 * mt + 1] * CH
                    if key0 + 512 <= w0:
                        continue
                    ps = kp.tile([128, 512], f32, tag="kps")
                    for k in range(8):
                        nc.tensor.matmul(
                            ps[:], (wkv_sb[:, k, mt * 128:(mt + 1) * 128]),
                            (xt_t[:, k, :]),
                            start=(k == 0), stop=(k == 7))
                    for gi in range(2):
                        g = 2 * mt + gi
                        wg0 = S - CHUNKS_G[g] * CH
                        lo = max(key0, wg0)
                        if lo >= key0 + 512:
                            continue
                        n = key0 + 512 - lo
                        rows = slice(gi * 64, gi * 64 + 64)
                        dst = slice(lo - wg0, lo - wg0 + n)
                        src = slice(lo - key0, lo - key0 + n)
                        nc.vector.tensor_copy(kdup[g][rows, dst], ps[rows, src])
                        # duplicate into the other partition half (SBUF->SBUF DMA)
                        orows = slice(64 - gi * 64, 128 - gi * 64)
                        nc.sync.dma_start(out=kdup[g][orows, dst],
                                          in_=kdup[g][rows, dst])
                # v rows for the 4 key chunks in this block
                for mi in (3, 2, 1, 0):
                    m = i5 * 4 + mi
                    ps = kp.tile([128, KV * HD], f32, tag="vps")
                    for k in range(8):
                        nc.tensor.matmul(
                            ps[:], (xt_t[:, k, mi * CH:(mi + 1) * CH]),
                            (wkv_sb[:, k, KV * HD:2 * KV * HD]),
                            start=(k == 0), stop=(k == 7))
                    for g in range(KV):
                        if m >= NCH - CHUNKS_G[g]:
                            ci = m - (NCH - CHUNKS_G[g])
                            nc.vector.tensor_copy(vext[g][:, ci, 0:HD],
                                                  ps[:, g * HD:(g + 1) * HD])
            for g in range(KV):
                nc.sync.dma_start(out=vext[g][:, :, HD:HD + 1],
                                  in_=ones_d[:, 0:CHUNKS_G[g]])

        # ---------------- phase B: attention ----------------
        wop = ctx.enter_context(tc.tile_pool(name="wop", bufs=1))
        wo_sb = wop.tile([128, 8, D], f32r)
        nc.sync.dma_start(out=wo_sb[:], in_=wo_d.rearrange("(k p) c -> p k c", p=128))
        with ExitStack() as actx:
            scp = actx.enter_context(tc.tile_pool(name="scp", bufs=2, space="PSUM"))
            osp = actx.enter_context(tc.tile_pool(name="osp", bufs=1, space="PSUM"))
            ptp = actx.enter_context(tc.tile_pool(name="ptp", bufs=3))
            nrm = actx.enter_context(tc.tile_pool(name="nrm", bufs=2))

            for p in range(8):
                heads = (2 * p, 2 * p + 1)
                g = p // 2
                outs = [osp.tile([HD + 1, QH], f32, tag=f"o{hi}", name=f"o{hi}p{p}")
                        for hi in range(2)]
                c0_pair = NCH - max(CHUNKS_H[h] for h in heads)
                for c in range(c0_pair, NCH):
                    for hi, h in enumerate(heads):
                        if c < NCH - CHUNKS_H[h]:
                            continue
                        rows = slice(hi * 64, hi * 64 + 64)
                        ci_g = c - (NCH - CHUNKS_G[g])
                        sc = scp.tile([128, QH], f32, tag="s")
                        for qc in range(2):
                            nc.tensor.matmul(
                                sc[:, qc * 512:(qc + 1) * 512],
                                (kdup[g][rows, ci_g * CH:(ci_g + 1) * CH]),
                                (qt[p][rows, qc * 512:(qc + 1) * 512]),
                                start=True, stop=True,
                                tile_position=(hi * 64, 0))
                        pt = ptp.tile([128, QH], f32r, tag="pt")
                        e = _ENTRIES[(h, c)]
                        nc.scalar.activation(pt[:], sc[:], Exp,
                                             bias=lnc_sb[:, e:e + 1], scale=1.0)
                        first = (c == NCH - CHUNKS_H[h])
                        for qc in range(2):
                            nc.tensor.matmul(
                                outs[hi][:, qc * 512:(qc + 1) * 512],
                                (vext[g][:, ci_g, :]),
                                (pt[:, qc * 512:(qc + 1) * 512]),
                                start=first, stop=(c == NCH - 1))
                # copy unnormalized out (+denom row) off PSUM fast, then
                # normalize rows 0..63 by row 64 into outst[p]
                for hi in range(2):
                    un = nrm.tile([HD + 1, QH], f32, tag="un", bufs=4)
                    nc.vector.tensor_copy(un[:], outs[hi][:])
                    dt_ = nrm.tile([128, QH // 128], f32, tag="dt")
                    nc.sync.dma_start(out=dt_[:], in_=un[HD:HD + 1, :])
                    rt = nrm.tile([128, QH // 128], f32, tag="rt")
                    nc.vector.reciprocal(rt[:], dt_[:])
                    rcp = nrm.tile([1, QH], f32, tag="rcp")
                    nc.sync.dma_start(out=rcp[:], in_=rt[:])
                    rcp_b = nrm.tile([64, QH], f32, tag="rcpb")
                    nc.gpsimd.partition_broadcast(rcp_b[:], rcp[0:1, :])
                    if hi == 0:
                        nc.vector.tensor_mul(outst[p][0:64, :],
                                             un[0:HD, :], rcp_b[:])
                    else:
                        tmp = nrm.tile([64, QH], f32r, tag="tmpB")
                        nc.vector.tensor_mul(tmp[:], un[0:HD, :], rcp_b[:])
                        nc.sync.dma_start(out=outst[p][64:128, :], in_=tmp[:])

        # ---------------- phase C: output projection ----------------
        with ExitStack() as octx:
            yp = octx.enter_context(tc.tile_pool(name="yp", bufs=2, space="PSUM"))
            yo = octx.enter_context(tc.tile_pool(name="yo", bufs=2))
            for mt in range(8):
                ps = yp.tile([128, QH], f32, tag="yps")
                for p in range(8):
                    for qc in range(2):
                        nc.tensor.matmul(
                            ps[:, qc * 512:(qc + 1) * 512],
                            (wo_sb[:, p, mt * 128:(mt + 1) * 128]),
                            (outst[p][:, qc * 512:(qc + 1) * 512]),
                            start=(p == 0), stop=(p == 7))
                ysb = yo.tile([128, QH], f32, tag="ysb")
                nc.vector.tensor_copy(ysb[:], ps[:])
                nc.sync.dma_start(out=yt_d[mt * 128:(mt + 1) * 128, :], in_=ysb[:])

    nc.compile()
    nc.m = get_hw_module(nc.m)
    return nc


def kernel(x, Wq, Wk, Wv, Wo):
    global _NC_CACHE, LAST_RESULT
    from concourse.bass_utils import run_bass_kernel_spmd

    if _NC_CACHE is None:
        _NC_CACHE = _build()
    nc = _NC_CACHE

    lnc = _lnc_table()
    wq_s = (Wq * (HD ** -0.5)).astype(np.float32)
    in_maps = []
    for core in range(N_CORES):
        b, half = divmod(core, 2)
        xt = np.ascontiguousarray(x[b].T.astype(np.float32))
        in_maps.append({
            "xt": xt,
            "xq": np.ascontiguousarray(xt[:, half * QH:(half + 1) * QH]),
            "wq": wq_s, "wk": Wk.astype(np.float32),
            "wv": Wv.astype(np.float32), "wo": Wo.astype(np.float32),
            "lnc": lnc,
            "ones": np.ones((CH, NCH), dtype=np.float32),
        })
    trace = bool(int(os.environ.get("KERNEL_TRACE", "0")))
    res = run_bass_kernel_spmd(nc, in_maps, list(range(N_CORES)), trace=trace)
    LAST_RESULT = res
    y = np.empty((B, S, D), dtype=np.float32)
    for core in range(N_CORES):
        b, half = divmod(core, 2)
        y[b, half * QH:(half + 1) * QH, :] = res.results[core]["yt"].T
    return y

